# revision 24
# baseline (speedup 1.0000x reference)
"""ALSH Conv2d kernel for 8 Trainium2 NeuronCores.

Strategy (data-parallel, host-routed sparsity):
  - The reference output is `dense_conv(x, W) * active_mask` where
    active[n, o] = (bucket(kernel_o) == bucket(sample_n)).  Masked-out
    channels are exactly zero, so we only compute the active channels.
  - Host computes the ALSH buckets in float64 (hash dots sit far from
    integer floor boundaries relative to f32 noise, so this matches the
    f32 reference bit-exactly), gathers each sample's active kernel rows,
    and packs them into <=128-channel "atoms".
  - 8 cores, 4 samples each, fixed atoms/sample-slot layout so all cores
    run one SPMD graph. Each atom runs a 3x3 conv as 18 accumulating
    matmuls (9 taps x 2 C-chunks of 128) over 7 pixel tiles of 448.
  - Host scatters atom outputs back into the zero-initialized full
    [32, 512, 56, 56] output.
"""

import os
import sys
import types

import numpy as np

N, C, H, W = 32, 256, 56, 56
O, KS = 512, 3
D = KS * KS * C  # 2304
TABLE_SIZE = 16
M_AUG = 3
NCORES = 8
S_PER_CORE = N // NCORES  # 4
HP, WP = H + 2, W + 2  # 58 padded
TILE_ROWS = 8
NT = H // TILE_ROWS  # 7 pixel tiles
TILE_PIX = TILE_ROWS * W  # 448
NJ = 2 * KS * KS  # 18 contraction chunks
CCH = C // 128  # 2 chunks of C

COMPUTE_DTYPE = os.environ.get("ALSH_DTYPE", "bf16")  # f32 | f32r | bf16
OUT_DTYPE = os.environ.get("ALSH_OUT_DTYPE", "bf16")  # f32 | bf16
IMPL = os.environ.get("ALSH_IMPL", "coltile")  # atom | coltile
SEG = 32  # channel segment width (PE column-group granularity)
GRP = 4  # segments packed per matmul slot via col tiling
XSLOTS = 8  # x-tile slots per core (a sample may occupy several)
XROW_SPLITS = [0, 12, 35, HP]  # x DMA row blocks so tile 0 can start early

_graph_cache = {}
last_exec_time_ns = None
last_results = None


def _install_patches():
    """Walrus in this container encodes at most 1 sync wait per CTRL
    instruction; Tile's kernel-tail drain can carry several. Split them
    across consecutive drains (same engine => same ordering semantics)."""
    import concourse.tile as tile
    import concourse.mybir as mybir
    from concourse.vector_clock import ScopedClock

    if getattr(tile.TileContext, "_alsh_patched", False):
        return

    def _patched(self, tick_clock, wait_clock):
        nc = self.nc
        drain_inst = nc.sync.drain()
        wait_clock.add_sem_waits(
            drain_inst.ins, ScopedClock({None: tick_clock.global_clock})
        )
        si = drain_inst.ins.sync_info
        waits = list(si.on_wait or []) if si is not None else []
        if len(waits) > 1:
            si.on_wait = waits[:1]
            for i in range(1, len(waits)):
                d2 = nc.sync.drain()
                if d2.ins.sync_info is None:
                    d2.ins.sync_info = mybir.SyncInfo(
                        on_wait=waits[i : i + 1], on_update=[]
                    )
                else:
                    d2.ins.sync_info.on_wait = waits[i : i + 1]
        nc.all_engine_barrier()
        assert self.sems is not None
        popped = nc._tile_sem_poison_stack.pop()
        assert popped is self._sem_poison
        nc.clear_and_free_semaphores(list(self.sems.allocated().values()))
        nc.all_engine_barrier()

    tile.TileContext._drain_and_barrier = _patched
    tile.TileContext._alsh_patched = True


def _split_excess_waits(nc, max_waits=1):
    """Walrus here encodes at most one sync wait per instruction. Hoist
    excess waits onto no-op carrier instructions inserted immediately
    before the overloaded instruction on the same engine (engines run
    their block instructions in order, so this is semantics-preserving)."""
    import bass_rust
    import concourse.mybir as mybir

    ctr = [0]

    def carrier(engine, waits):
        ctr[0] += 1
        nop = bass_rust.InstNoOp(name=f"WSPLIT-{ctr[0]}", engine=engine)
        nop.sync_info = mybir.SyncInfo(on_wait=list(waits), on_update=[])
        return nop

    n_split = 0
    for fn in nc.m.functions:
        for bb in fn.blocks:
            out = []
            for inst in bb.instructions:
                si = inst.sync_info
                if si is not None and si.on_wait and len(si.on_wait) > max_waits:
                    waits = list(si.on_wait)
                    si.on_wait = waits[-max_waits:]
                    extra = waits[: -max_waits]
                    for i in range(0, len(extra), max_waits):
                        out.append(carrier(inst.engine, extra[i : i + max_waits]))
                    n_split += 1
                out.append(inst)
            bb.instructions[:] = out
    return n_split


def _install_trace_hook():
    try:
        from antenv import axon_hooks  # noqa: F401
        return
    except ImportError:
        pass
    try:
        from trn_agent_boot.trn_boot import _ntff_profile_via_ctypes
    except ImportError:
        return
    hook = _ntff_profile_via_ctypes("/opt/axon/libaxon_pjrt.so")
    m = types.ModuleType("antenv.axon_hooks")
    m.get_axon_ntff_profile_hook = lambda: hook
    m.set_axon_ntff_profile_hook = lambda h: None
    sys.modules["antenv.axon_hooks"] = m
    import antenv

    antenv.axon_hooks = m


def _bucket64(dots):
    return np.mod(np.abs(np.floor(dots)), TABLE_SIZE).astype(np.int32)


def _routing(x, kernels, hash_a):
    """Replicate the reference hashing in float64 on host."""
    a_main = hash_a[:D].astype(np.float64)
    a_aug = hash_a[D:].astype(np.float64)
    k64 = kernels.astype(np.float64)
    n2 = np.sum(k64 * k64, axis=1)
    powers = np.stack([n2 ** (2 ** i) for i in range(M_AUG)], axis=1)
    k_dots = k64 @ a_main + powers @ a_aug
    k_bucket = _bucket64(k_dots)

    q = x.astype(np.float64).mean(axis=(2, 3))  # [N, C]
    q_t = np.tile(q, (1, KS * KS))  # [N, D]
    q_dots = q_t @ a_main + 0.5 * np.sum(a_aug)
    q_bucket = _bucket64(q_dots)
    return k_bucket, q_bucket


def _mybir_dtype(mybir):
    return {
        "f32": mybir.dt.float32,
        "f32r": mybir.dt.float32r,
        "bf16": mybir.dt.bfloat16,
    }[COMPUTE_DTYPE]


def _np_in_dtype():
    if COMPUTE_DTYPE == "bf16":
        import ml_dtypes

        return ml_dtypes.bfloat16
    return np.float32


def _build_graph(a_slots):
    """Build the SPMD Bass graph for one core. a_slots[s] = number of
    weight atoms processed against local sample s."""
    import concourse.bass as bass
    import concourse.mybir as mybir
    import concourse.tile as tile

    A = sum(a_slots)
    dt_in = _mybir_dtype(mybir)
    f32 = mybir.dt.float32
    dt_out = mybir.dt.bfloat16 if OUT_DTYPE == "bf16" else f32

    nc = bass.Bass()
    xs_ext = nc.declare_dram_parameter(
        "xs", [S_PER_CORE, CCH, 128, HP, WP], dt_in, isOutput=False
    )
    ws_ext = nc.declare_dram_parameter(
        "ws", [A, 128, NJ, 128], dt_in, isOutput=False
    )
    out_ext = nc.declare_dram_parameter(
        "out", [A, 128, NT * TILE_PIX], dt_out, isOutput=True
    )

    with tile.TileContext(nc) as tc:
        with (
            tc.tile_pool(name="xp", bufs=2) as xpool,
            tc.tile_pool(name="wp", bufs=3) as wpool,
            tc.tile_pool(name="op", bufs=2) as opool,
            tc.tile_pool(name="pp", bufs=4, space="PSUM") as ppool,
        ):
            a = 0
            for s in range(S_PER_CORE):
                xt = [
                    xpool.tile(
                        [128, HP, WP], dt_in, tag=f"x{c2}", name=f"xt{c2}"
                    )
                    for c2 in range(CCH)
                ]
                # Row-blocked loads so tile 0's matmuls start after a small
                # fraction of x has landed (matters for the first sample).
                for r0, r1 in zip(XROW_SPLITS, XROW_SPLITS[1:]):
                    for c2 in range(CCH):
                        nc.sync.dma_start(xt[c2][:, r0:r1], xs_ext[s, c2, :, r0:r1])
                for _k in range(a_slots[s]):
                    wt = wpool.tile([128, NJ, 128], dt_in, tag="w")
                    # split so the c2=0 half (needed first) arrives first
                    nc.sync.dma_start(wt[:, : NJ // 2], ws_ext[a, :, : NJ // 2])
                    nc.sync.dma_start(wt[:, NJ // 2 :], ws_ext[a, :, NJ // 2 :])
                    ot = opool.tile([128, NT, TILE_ROWS, W], dt_out, tag="o")
                    for t in range(NT):
                        r0 = t * TILE_ROWS
                        pt = ppool.tile([128, TILE_ROWS, W], f32, tag="ps")
                        j = 0
                        for c2 in range(CCH):
                            for kh in range(KS):
                                for kw in range(KS):
                                    rhs = xt[c2][
                                        :, r0 + kh : r0 + kh + TILE_ROWS, kw : kw + W
                                    ]
                                    nc.tensor.matmul(
                                        pt[:],
                                        wt[:, c2 * KS * KS + kh * KS + kw, :],
                                        rhs,
                                        start=(j == 0),
                                        stop=(j == NJ - 1),
                                    )
                                    j += 1
                        nc.vector.tensor_copy(ot[:, t], pt[:])
                    nc.sync.dma_start(out_ext[a], ot[:].rearrange("p a b c -> p (a b c)"))
                    a += 1
    return nc


def _build_graph_ct(plan):
    """Col-tiled SPMD graph. plan[i] = tuple of local-sample indices, one
    per 32-wide column group of matmul slot i (identical across cores).
    Each slot computes up to GRP*SEG output channels: group g's weights sit
    in columns [32g, 32g+32) of the slot's weight tile and write PSUM
    partitions [32g, 32g+32) via tile_position."""
    import concourse.bass as bass
    import concourse.mybir as mybir
    import concourse.tile as tile

    NSLOT = len(plan)
    dt_in = _mybir_dtype(mybir)
    f32 = mybir.dt.float32
    dt_out = mybir.dt.bfloat16 if OUT_DTYPE == "bf16" else f32

    nc = bass.Bass()
    xs_ext = nc.declare_dram_parameter(
        "xs", [XSLOTS, CCH, 128, HP, WP], dt_in, isOutput=False
    )
    ws_ext = nc.declare_dram_parameter(
        "ws", [NSLOT, 128, NJ, 128], dt_in, isOutput=False
    )
    out_ext = nc.declare_dram_parameter(
        "out", [NSLOT, 128, NT * TILE_PIX], dt_out, isOutput=True
    )

    with tile.TileContext(nc) as tc:
        with (
            tc.tile_pool(name="xp", bufs=1) as xpool,
            tc.tile_pool(name="wp", bufs=3) as wpool,
            tc.tile_pool(name="op", bufs=2) as opool,
            tc.tile_pool(name="pp", bufs=4, space="PSUM") as ppool,
        ):
            used = sorted({s for grp in plan for s in grp})
            xt = {}
            for s in used:
                xt[s] = [
                    xpool.tile(
                        [128, HP, WP], dt_in, tag=f"x{s}_{c2}", name=f"xt{s}_{c2}"
                    )
                    for c2 in range(CCH)
                ]

            def dma_x(s, dual=False):
                for r0, r1 in zip(XROW_SPLITS, XROW_SPLITS[1:]):
                    for c2 in range(CCH):
                        eng = nc.gpsimd if (dual and c2 == 1) else nc.sync
                        eng.dma_start(xt[s][c2][:, r0:r1], xs_ext[s, c2, :, r0:r1])

            # PE warmup: matmuls on zeroed scratch keep the PE busy (HAM
            # warm, IRAM fetched) while the first real DMAs land. Reuses
            # existing pool tags so tile/bank allocation is unchanged.
            n_warm = int(os.environ.get("ALSH_WARMUP", "30"))
            if n_warm:
                wsc = opool.tile(
                    [128, NT, TILE_ROWS, W], dt_out, tag="o", name="wsc"
                )
                nc.vector.memset(wsc[:, 0:2], 0.0)
                wlhs = wsc[:, 0].rearrange("p a b -> p (a b)")[:, :128]
                wps = ppool.tile([128, TILE_ROWS, W], f32, tag="ps", name="wps")
                for wi in range(n_warm):
                    nc.tensor.matmul(
                        wps[:], wlhs, wsc[:, 1], start=True, stop=True
                    )

            first_samples = sorted(set(plan[0]))
            for s in first_samples:
                dma_x(s, dual=True)
            # weights go on the gpsimd (SWDGE) queue so they never queue
            # behind the long x loads on the sync queue
            weng = nc.gpsimd if os.environ.get("ALSH_WQ", "gpsimd") == "gpsimd" else nc.sync
            wt0 = wpool.tile([128, NJ, 128], dt_in, tag="w", name="wt0")
            weng.dma_start(wt0[:, : NJ // 2], ws_ext[0, :, : NJ // 2])
            weng.dma_start(wt0[:, NJ // 2 :], ws_ext[0, :, NJ // 2 :])
            for s in used:
                if s not in first_samples:
                    dma_x(s)

            for i, grp in enumerate(plan):
                if i == 0:
                    wt = wt0
                else:
                    wt = wpool.tile([128, NJ, 128], dt_in, tag="w", name=f"wt{i}")
                    weng.dma_start(wt[:, : NJ // 2], ws_ext[i, :, : NJ // 2])
                    weng.dma_start(wt[:, NJ // 2 :], ws_ext[i, :, NJ // 2 :])
                ot = opool.tile([128, NT, TILE_ROWS, W], dt_out, tag="o", name=f"ot{i}")
                for t in range(NT):
                    r0 = t * TILE_ROWS
                    pt = ppool.tile(
                        [128, TILE_ROWS, W], f32, tag="ps", name=f"pt{i}_{t}"
                    )
                    for j in range(NJ):
                        c2, tap = divmod(j, KS * KS)
                        kh, kw = divmod(tap, KS)
                        rhs_all = {}
                        for g, s_loc in enumerate(grp):
                            if s_loc not in rhs_all:
                                rhs_all[s_loc] = xt[s_loc][c2][
                                    :, r0 + kh : r0 + kh + TILE_ROWS, kw : kw + W
                                ]
                            nc.tensor.matmul(
                                pt[SEG * g : SEG * (g + 1)],
                                wt[:, j, SEG * g : SEG * (g + 1)],
                                rhs_all[s_loc],
                                start=(j == 0),
                                stop=(j == NJ - 1),
                                tile_position=(0, SEG * g),
                                skip_group_check=True,
                            )
                    nc.vector.tensor_copy(ot[:, t], pt[:])
                nc.sync.dma_start(out_ext[i], ot[:].rearrange("p a b c -> p (a b c)"))
    return nc


def kernel(x, kernels, hash_a, mode=None):
    x = np.ascontiguousarray(np.asarray(x, dtype=np.float32))
    kernels = np.ascontiguousarray(np.asarray(kernels, dtype=np.float32))
    hash_a = np.asarray(hash_a, dtype=np.float32)

    k_bucket, q_bucket = _routing(x, kernels, hash_a)

    # Per-sample active channel lists.
    idx_lists = [np.where(k_bucket == q_bucket[n])[0] for n in range(N)]
    gran = SEG if IMPL == "coltile" else 128
    units_of = [int(-(-len(ix) // gran)) for ix in idx_lists]  # ceil

    # Assign samples to cores: sort by unit count desc, snake over cores.
    order = sorted(range(N), key=lambda n: (-units_of[n], n))
    core_samples = [[] for _ in range(NCORES)]
    for i, n in enumerate(order):
        blk, pos = divmod(i, NCORES)
        c = pos if blk % 2 == 0 else NCORES - 1 - pos
        core_samples[c].append(n)
    # Within each core sort samples desc so slot k has the max unit count
    # across cores (uniform static graph).
    for c in range(NCORES):
        core_samples[c].sort(key=lambda n: (-units_of[n], n))
    u_slots = [
        max(units_of[core_samples[c][s]] for c in range(NCORES))
        for s in range(S_PER_CORE)
    ]

    out_full = np.zeros((N, O, H, W), dtype=np.float32)
    if all(len(ix) == 0 for ix in idx_lists):
        return out_full

    np_in = _np_in_dtype()
    kern4 = kernels.reshape(O, KS * KS, CCH, 128)  # [o, tap, c2, c]
    in_maps = []
    scatter = []  # per core: list of (slot, row0, sample, channel_indices)

    if IMPL == "coltile":
        # Segment-level packing with 8 x-slots of fixed (uniform across
        # cores) segment capacities. A sample's segments may split across
        # several x-slots; its padded x is then duplicated per slot.
        core_bundles = [
            [(n, units_of[n]) for n in core_samples[c] if units_of[n] > 0]
            for c in range(NCORES)
        ]
        max_segs = max(1, max(sum(b for _, b in cb) for cb in core_bundles))
        NSLOT = -(-max_segs // GRP)

        def caps_for(nslot):
            tot = nslot * GRP
            base, extra = divmod(tot, XSLOTS)
            return [base + 1] * extra + [base] * (XSLOTS - extra)

        def pack_core(bundles, caps):
            """Assign each (sample, nsegs) a disjoint set of x-slots whose
            caps cover nsegs. Returns per-xslot (sample, seg_start, cnt)
            or None if infeasible."""
            slots_alloc = [None] * len(caps)
            order_slots = sorted(range(len(caps)), key=lambda i: -caps[i])
            free = list(order_slots)
            res = [None] * len(caps)
            for n, nsegs in sorted(bundles, key=lambda t: -t[1]):
                got, covered = [], 0
                while covered < nsegs:
                    if not free:
                        return None
                    sl = free.pop(0)
                    got.append(sl)
                    covered += caps[sl]
                seg0 = 0
                for sl in got:
                    cnt = min(caps[sl], nsegs - seg0)
                    res[sl] = (n, seg0, cnt)
                    seg0 += cnt
            return res

        while True:
            caps = caps_for(NSLOT)
            packs = [pack_core(cb, caps) for cb in core_bundles]
            if all(p is not None for p in packs):
                break
            NSLOT += 1

        refs = []
        for xs_i in range(XSLOTS):
            refs += [xs_i] * caps[xs_i]
        plan = [tuple(refs[i * GRP : (i + 1) * GRP]) for i in range(NSLOT)]

        for c in range(NCORES):
            xs = np.zeros((XSLOTS, CCH, 128, HP, WP), dtype=np_in)
            ws = np.zeros((NSLOT, 128, NJ, 128), dtype=np_in)
            sc = []
            seg_cursor = [0] * XSLOTS  # groups consumed per x-slot so far
            for xs_i, alloc in enumerate(packs[c]):
                if alloc is not None:
                    n = alloc[0]
                    xs[xs_i, :, :, 1 : H + 1, 1 : W + 1] = x[n].reshape(
                        CCH, 128, H, W
                    )
            for i, grp in enumerate(plan):
                for g, xs_i in enumerate(grp):
                    alloc = packs[c][xs_i]
                    k_local = seg_cursor[xs_i]
                    seg_cursor[xs_i] += 1
                    if alloc is None:
                        continue
                    n, seg0, cnt = alloc
                    if k_local >= cnt:
                        continue
                    k = seg0 + k_local
                    chans = idx_lists[n][k * SEG : (k + 1) * SEG]
                    if len(chans):
                        blk = kern4[chans]
                        blk = blk.transpose(3, 2, 1, 0).reshape(128, NJ, len(chans))
                        ws[i, :, :, SEG * g : SEG * g + len(chans)] = blk
                        sc.append((i, SEG * g, n, chans))
            in_maps.append({"xs": xs, "ws": ws})
            scatter.append(sc)
        key = ("ct", tuple(plan), COMPUTE_DTYPE, OUT_DTYPE)
        builder = lambda: _build_graph_ct(plan)
    else:
        u_slots = [max(k, 1) for k in u_slots]
        A = sum(u_slots)
        for c in range(NCORES):
            xs = np.zeros((S_PER_CORE, CCH, 128, HP, WP), dtype=np_in)
            ws = np.zeros((A, 128, NJ, 128), dtype=np_in)
            sc = []
            a = 0
            for s in range(S_PER_CORE):
                n = core_samples[c][s]
                xs[s, :, :, 1 : H + 1, 1 : W + 1] = x[n].reshape(CCH, 128, H, W)
                ix = idx_lists[n]
                for k in range(u_slots[s]):
                    chans = ix[k * 128 : (k + 1) * 128]
                    if len(chans):
                        blk = kern4[chans]
                        blk = blk.transpose(3, 2, 1, 0).reshape(128, NJ, len(chans))
                        ws[a, :, :, : len(chans)] = blk
                        sc.append((a, 0, n, chans))
                    a += 1
            in_maps.append({"xs": xs, "ws": ws})
            scatter.append(sc)
        key = ("atom", tuple(u_slots), COMPUTE_DTYPE, OUT_DTYPE)
        builder = lambda: _build_graph(u_slots)

    # Build / fetch graph and run.
    _install_patches()
    if key not in _graph_cache:
        nc_new = builder()
        _split_excess_waits(nc_new)
        _graph_cache[key] = nc_new
    nc = _graph_cache[key]

    trace = bool(int(os.environ.get("ALSH_TRACE", "0")))
    if trace:
        _install_trace_hook()
        import concourse.bass_utils as bu

        bu.upload_artifacts = lambda d: d

    from concourse.bass_utils import run_bass_kernel_spmd

    res = run_bass_kernel_spmd(
        nc, in_maps, list(range(NCORES)), trace=trace
    )
    global last_exec_time_ns, last_results
    last_exec_time_ns = res.exec_time_ns
    last_results = res

    for c in range(NCORES):
        out_c = np.asarray(res.results[c]["out"], dtype=np.float32)
        for (i, r0, n, chans) in scatter[c]:
            out_full[n, chans] = out_c[i, r0 : r0 + len(chans)].reshape(
                len(chans), H, W
            )
    return out_full
